# revision 35
# baseline (speedup 1.0000x reference)
"""BitNetLinear on 8 Trainium2 NeuronCores.

Computes out = x @ sign(weight).T + bias for x[4,2048,4096] f32,
weight[4096,4096] f32, bias[4096] f32.

Strategy: 2-way data parallel over rows x 4-way tensor parallel over
out_features (each core owns a [4096, 1024] block of the [8192, 4096]
output; no collectives, host stitches blocks).

Per core a single all-fp8 DoubleRow stream. x splits hi/lo:
  hi = e4m3(x) over all 4096 dims, matched with weights sign(w) (exact
       in e4m3);
  lo = e4m3(32*(x - hi)) over the first LO_DP*256 dims, matched with
       weights sign(w)/32 (+-2^-5, also exact in e4m3).
Both are concatenated into one K' = (16+LO_DP)*256 contraction stream
of DoubleRow matmuls accumulating into the same fp32 PSUM banks, so
the PE never switches weight-path modes. DoubleRow processes 2 fp8
rows/cycle (HW-measured ~2x fp16 here with LDWEIGHTS hidden), so this
costs (16+LO_DP)/32 of a full fp16 pass. With LO_DP=9 the hi-only
tail dims (2304..4095) leave rel-l2 1.756e-2 / rel-max 1.749e-2
(numpy-validated against f64 and confirmed on HW to 4 digits), inside
the 2e-2 gate; dims covered by lo are ~1e-3-exact.

Layouts are precomputed on the host so every DMA is contiguous. All
weights stay resident in SBUF (54KB per partition; only the hi blocks
are DMAed -- the lo blocks are hi/32, derived by the otherwise-idle
DVE), x tiles stream per m-tile, and each [128, 512] output chunk
accumulates DPT matmuls before a DVE eviction fused with the bias
add. The first ST m-tiles run jointly, k-block-major, so PE
consumption paces the ~7 MB weight+startup preload; the leading DMA
groups are small so the first matmul starts early, and a burst of
throwaway matmuls on zeroed scratch warms the PE's HAM allocation
during the initial DMA wait.
"""

import sys
import types

import numpy as np

import concourse.mybir as mybir
import concourse.tile as tile
from concourse import bacc
from concourse.bass_utils import run_bass_kernel_spmd


def _ensure_axon_hooks():
    """run_bass_kernel_spmd(trace=True) (or BASS_TRACE=1 in the env) imports
    antenv.axon_hooks, which some agent images lack. Provide it, and register
    the ctypes NTFF hook if the boot shim is available, so tracing works (or
    degrades to a warning) instead of crashing."""
    try:
        import antenv.axon_hooks  # noqa: F401

        return
    except ImportError:
        pass
    m = types.ModuleType("antenv.axon_hooks")
    m._h = None
    m.set_axon_ntff_profile_hook = lambda h: setattr(m, "_h", h)
    m.get_axon_ntff_profile_hook = lambda: m._h
    sys.modules["antenv.axon_hooks"] = m
    try:
        import antenv

        antenv.axon_hooks = m
    except ImportError:
        pass
    try:
        from trn_agent_boot.trn_boot import _ntff_profile_via_ctypes

        m.set_axon_ntff_profile_hook(
            _ntff_profile_via_ctypes("/opt/axon/libaxon_pjrt.so")
        )
    except Exception:
        pass


_ensure_axon_hooks()

B, S, D_IN, D_OUT = 4, 2048, 4096, 4096
M_TOT = B * S  # 8192
N_CORES = 8
MG, OG = 2, 4  # data-parallel row groups x tensor-parallel out_feature groups
M_SH = M_TOT // MG  # 4096 rows per core
O_SH = D_OUT // OG  # 1024 out features per core
P = 128
HI_DP = D_IN // (2 * P)  # 16 hi pair-blocks of 256 contraction dims
LO_DP = 9  # lo pair-blocks (residual coverage of the first 2304 dims)
LO_K = LO_DP * 2 * P
DPT = HI_DP + LO_DP  # 27 pair-blocks in the fused stream
LO_SCALE = 32.0  # lo values x32, lo weights /32 (both exact in e4m3)
MT = M_SH // P  # 32 m-tiles per core
NF = 512  # psum free dim per matmul chunk (one PSUM bank of fp32)
NCH = O_SH // NF  # 2 output chunks per m-tile
ST = 3  # m-tiles processed jointly in the startup phase
GROUPS = [1, 2, 4, 4, 4, 4, 4, 2]  # x pair-blocks per startup DMA (sum = DPT)
assert sum(GROUPS) == DPT
WGROUPS = [1, 2, 4, 4, 5]  # hi weight pair-blocks per startup DMA (sum = 16)
assert sum(WGROUPS) == HI_DP
W2 = 2 * P  # free width of one x pair-block (h*128 + m)
WO = 2 * O_SH  # free width of one w pair-block (h*1024 + o)

_CACHE = {}


def _build():
    nc = bacc.Bacc("TRN2", target_bir_lowering=False, debug=False)
    f8, f32 = mybir.dt.float8e4, mybir.dt.float32

    # steady-state x, one m-tile per row: partition = d,
    # free = pb*256 + h*128 + m (DoubleRow pair layout)
    x_d = nc.dram_tensor("x8", [MT, P, DPT * W2], f8, kind="ExternalInput")
    # startup copies of m-tiles 0..ST-1, k-block-major, grouped for DMA:
    # free = pb*(ST*256) + st*256 + h*128 + m
    xs_d = nc.dram_tensor("xs8", [P, DPT * ST * W2], f8, kind="ExternalInput")
    # hi weights only: free = pb*2048 + h*1024 + o (lo weights = hi/32 are
    # derived on-chip by the otherwise-idle DVE)
    w_d = nc.dram_tensor("w8", [P, HI_DP * WO], f8, kind="ExternalInput")
    bias_d = nc.dram_tensor("biasb", [P, O_SH], f32, kind="ExternalInput")
    out_d = nc.dram_tensor("out", [M_SH, O_SH], f32, kind="ExternalOutput")

    DR = mybir.MatmulPerfMode.DoubleRow

    with tile.TileContext(nc) as tc:
        with (
            tc.tile_pool(name="wpool", bufs=1) as wpool,
            tc.tile_pool(name="xpool", bufs=4) as xpool,
            tc.tile_pool(name="psum", bufs=4, space="PSUM") as psum_pool,
        ):

            def load_x(mt, eng=None):
                x_t = xpool.tile([P, DPT * W2], f8, name="x", tag="x")
                (eng or nc.sync).dma_start(out=x_t[:], in_=x_d[mt])
                return x_t

            def alloc_psums():
                return [
                    psum_pool.tile([P, NF], f32, name=f"ps{oc}", tag=f"ps{oc}")
                    for oc in range(NCH)
                ]

            def mm(psums, x_pair, pb, oc):
                # x_pair: [P, 2, 128] fp8 pair view of one pair-block
                nc.tensor.matmul(
                    psums[oc][:],
                    x_pair,
                    w_sb[pb].rearrange("p (h o) -> p h o", h=2)[
                        :, :, oc * NF : (oc + 1) * NF
                    ],
                    start=pb == 0,
                    stop=pb == DPT - 1,
                    perf_mode=DR,
                )

            def evict(opool, mt, psums, ocs=None):
                for oc in ocs if ocs is not None else range(NCH):
                    o_sb = opool.tile([P, NF], f32, name="o_sb", tag=f"o{oc}")
                    nc.vector.tensor_add(
                        o_sb[:], psums[oc][:], bias_sb[:, oc * NF : (oc + 1) * NF]
                    )
                    nc.scalar.dma_start(
                        out=out_d[mt * P : (mt + 1) * P, oc * NF : (oc + 1) * NF],
                        in_=o_sb[:],
                    )

            w_sb = []  # per-pair-block [P, WO] views
            with tc.tile_pool(name="xstart", bufs=1) as xstart_pool:
                # HAM warmup: the PE's SBUF-port allocation ramps to 8/8 only
                # after several us of continuous activity (early matmuls issue
                # at ~534ns vs 220ns steady). Burn the initial DMA-wait window
                # with throwaway DoubleRow matmuls on zeroed scratch; they
                # land in psum set 0, which the real m-tile-0 group resets
                # via start=True.
                psums_st = [alloc_psums() for _ in range(ST)]
                warm = xstart_pool.tile([P, 2 * NF], f8, name="warm")
                nc.vector.memset(warm[:], 0)
                for _ in range(14):
                    nc.tensor.matmul(
                        psums_st[0][0][:],
                        warm[:, : 2 * P].rearrange("p (h m) -> p h m", h=2),
                        warm[:].rearrange("p (h o) -> p h o", h=2),
                        start=True,
                        stop=True,
                        perf_mode=DR,
                    )

                # startup x (m-tiles 0..ST-1) k-block-major plus the weight
                # stream, interleaved so each group lands as the PE needs it;
                # leading groups are small so the first matmul starts early
                # (xs before w: LDWEIGHTS loads the stationary x, so the xs
                # block gates the tensor queue ahead of the weights)
                xs_sb = []  # per-pair-block [P, ST*W2] views
                xoff = woff = 0
                wg = list(WGROUPS)
                for gi, gsz in enumerate(GROUPS):
                    t = xstart_pool.tile(
                        [P, gsz * ST * W2], f8, name=f"xs{gi}", tag=f"xs{gi}"
                    )
                    # early groups on the sync ring; late groups (needed
                    # tens of us in) ride the scalar ring once the weight
                    # stream there has drained, halving sync-ring pressure
                    xeng = nc.sync if gi < 5 else nc.scalar
                    xeng.dma_start(
                        out=t[:],
                        in_=xs_d[:, xoff * ST * W2 : (xoff + gsz) * ST * W2],
                    )
                    for i in range(gsz):
                        xs_sb.append(t[:, i * ST * W2 : (i + 1) * ST * W2])
                    xoff += gsz
                    if wg:
                        wsz = wg.pop(0)
                        w = wpool.tile(
                            [P, wsz * WO], f8, name=f"w_{gi}", tag=f"w_{gi}"
                        )
                        nc.sync.dma_start(
                            out=w[:], in_=w_d[:, woff * WO : (woff + wsz) * WO]
                        )
                        for i in range(wsz):
                            w_sb.append(w[:, i * WO : (i + 1) * WO])
                        woff += wsz
                bias_sb = wpool.tile([P, O_SH], f32, name="bias_sb")
                nc.scalar.dma_start(out=bias_sb[:], in_=bias_d[:])

                # derive the lo weight blocks (hi/32, exact in e4m3) on the
                # DVE, which sits idle until the first evictions
                for lp in range(LO_DP):
                    wl = wpool.tile([P, WO], f8, name=f"wlo{lp}", tag=f"wlo{lp}")
                    nc.vector.tensor_scalar_mul(
                        wl[:], w_sb[lp], 1.0 / LO_SCALE
                    )
                    w_sb.append(wl)

                # prefetch steady-state x ahead of the startup evictions
                # (in-order sync stream: later dma_starts would head-of-line
                # block behind eviction DMAs otherwise)
                x_next = {mt: load_x(mt) for mt in range(ST, ST + 3)}

                # startup: ST m-tiles jointly, k-block-major, paced by the
                # weight stream
                for pb in range(DPT):
                    for st in range(ST):
                        xp = xs_sb[pb][
                            :, st * W2 : (st + 1) * W2
                        ].rearrange("p (h m) -> p h m", h=2)
                        for oc in range(NCH):
                            mm(psums_st[st], xp, pb, oc)

            with tc.tile_pool(name="opool", bufs=2) as opool:
                for st in range(ST):
                    evict(opool, st, psums_st[st])

                for mt in range(ST, MT):
                    x_t = x_next.pop(mt) if mt in x_next else load_x(mt)
                    psums = alloc_psums()
                    if mt < MT - 1:
                        for pb in range(DPT):
                            xp = x_t[
                                :, pb * W2 : (pb + 1) * W2
                            ].rearrange("p (h m) -> p h m", h=2)
                            for oc in range(NCH):
                                mm(psums, xp, pb, oc)
                        evict(opool, mt, psums)
                    else:
                        # last m-tile: oc-major so each output chunk finishes
                        # and evicts as early as possible; the final chunk
                        # drains in two half-width pieces to shorten the tail
                        for oc in range(NCH):
                            for pb in range(DPT):
                                xp = x_t[
                                    :, pb * W2 : (pb + 1) * W2
                                ].rearrange("p (h m) -> p h m", h=2)
                                mm(psums, xp, pb, oc)
                            if oc < NCH - 1:
                                evict(opool, mt, psums, ocs=[oc])
                            else:
                                for h in range(2):
                                    hf = NF // 2
                                    c0 = oc * NF + h * hf
                                    o_sb = opool.tile(
                                        [P, hf], f32, name="o_sb", tag=f"ot{h}"
                                    )
                                    nc.vector.tensor_add(
                                        o_sb[:],
                                        psums[oc][:, h * hf : (h + 1) * hf],
                                        bias_sb[:, c0 : c0 + hf],
                                    )
                                    # second piece on the idle sync ring so
                                    # the two final issues parallelize
                                    eng = nc.scalar if h == 0 else nc.sync
                                    eng.dma_start(
                                        out=out_d[
                                            mt * P : (mt + 1) * P, c0 : c0 + hf
                                        ],
                                        in_=o_sb[:],
                                    )
    nc.compile()
    return nc


def _prep_inputs(x, weight, bias):
    import ml_dtypes

    f8 = ml_dtypes.float8_e4m3
    x = np.asarray(x, dtype=np.float32)
    weight = np.asarray(weight, dtype=np.float32)
    bias = np.asarray(bias, dtype=np.float32)

    xf = np.ascontiguousarray(x.reshape(M_TOT, D_IN))
    x_hi = xf.astype(f8)
    res = xf - x_hi.astype(np.float32)
    x_lo = (res[:, :LO_K] * LO_SCALE).astype(f8)
    xcat = np.concatenate([x_hi, x_lo], axis=1)  # [M_TOT, DPT*256] f8

    qw = np.sign(weight)  # [o, d] f32

    # per o-group weights + broadcast bias, shared by cores in the group
    w_og, bias_og = [], []
    for og in range(OG):
        o0 = og * O_SH
        blk = np.ascontiguousarray(qw[o0 : o0 + O_SH, :].T)  # [d, o] f32
        # hi blocks only: [HI_DP, d, h*O_SH + o] -> grouped [P, HI_DP*WO]
        w8 = (
            blk.astype(f8)
            .reshape(HI_DP, 2, P, O_SH)
            .transpose(0, 2, 1, 3)
            .reshape(HI_DP, P, WO)
        )
        w_og.append(
            np.ascontiguousarray(w8.transpose(1, 0, 2)).reshape(P, HI_DP * WO)
        )
        bias_og.append(
            np.ascontiguousarray(
                np.broadcast_to(bias[o0 : o0 + O_SH], (P, O_SH))
            )
        )

    # per m-group x layouts, shared by cores in the group
    x_mg, xs_mg = [], []
    for mg in range(MG):
        m0 = mg * M_SH
        # steady state: [mt, d, pb*256 + h*128 + m]
        r = xcat[m0 : m0 + M_SH].reshape(MT, P, DPT, 2, P)  # [mt,m,pb,h,d]
        xt = np.ascontiguousarray(r.transpose(0, 4, 2, 3, 1)).reshape(
            MT, P, DPT * W2
        )
        x_mg.append(xt)
        # startup copies, k-block-major over the first ST m-tiles:
        # [pb, d, st*256 + h*128 + m] -> grouped [P, DPT*ST*W2]
        xs = np.empty((DPT, P, ST * W2), dtype=f8)
        for st in range(ST):
            xs[:, :, st * W2 : (st + 1) * W2] = (
                xt[st].reshape(P, DPT, W2).transpose(1, 0, 2)
            )
        xs_mg.append(
            np.ascontiguousarray(xs.transpose(1, 0, 2)).reshape(
                P, DPT * ST * W2
            )
        )

    in_maps = []
    for c in range(N_CORES):
        mg, og = c // OG, c % OG
        in_maps.append(
            {
                "x8": x_mg[mg],
                "xs8": xs_mg[mg],
                "w8": w_og[og],
                "biasb": bias_og[og],
            }
        )
    return in_maps


def run(inputs, trace=False):
    """Run the SPMD kernel; returns (full_output, BassKernelResults)."""
    if "nc" not in _CACHE:
        _CACHE["nc"] = _build()
    nc = _CACHE["nc"]
    in_maps = _prep_inputs(inputs["x"], inputs["weight"], inputs["bias"])
    res = run_bass_kernel_spmd(nc, in_maps, list(range(N_CORES)), trace=trace)
    out = np.empty((M_TOT, D_OUT), dtype=np.float32)
    for c in range(N_CORES):
        mg, og = c // OG, c % OG
        out[mg * M_SH : (mg + 1) * M_SH, og * O_SH : (og + 1) * O_SH] = res.results[
            c
        ]["out"]
    return out.reshape(B, S, D_OUT), res


def kernel(x, weight, bias):
    out, _ = run({"x": x, "weight": weight, "bias": bias})
    return out


# revision 36
# speedup vs baseline: 1.0166x; 1.0166x over previous
"""BitNetLinear on 8 Trainium2 NeuronCores.

Computes out = x @ sign(weight).T + bias for x[4,2048,4096] f32,
weight[4096,4096] f32, bias[4096] f32.

Strategy: 2-way data parallel over rows x 4-way tensor parallel over
out_features (each core owns a [4096, 1024] block of the [8192, 4096]
output; no collectives, host stitches blocks).

Per core a single all-fp8 DoubleRow stream. x splits hi/lo:
  hi = e4m3(x) over all 4096 dims, matched with weights sign(w) (exact
       in e4m3);
  lo = e4m3(32*(x - hi)) over the first LO_DP*256 dims, matched with
       weights sign(w)/32 (+-2^-5, also exact in e4m3).
Both are concatenated into one K' = (16+LO_DP)*256 contraction stream
of DoubleRow matmuls accumulating into the same fp32 PSUM banks, so
the PE never switches weight-path modes. DoubleRow processes 2 fp8
rows/cycle (HW-measured ~2x fp16 here with LDWEIGHTS hidden), so this
costs (16+LO_DP)/32 of a full fp16 pass. With LO_DP=9 the hi-only
tail dims (2304..4095) leave rel-l2 1.756e-2 / rel-max 1.749e-2
(numpy-validated against f64 and confirmed on HW to 4 digits), inside
the 2e-2 gate; dims covered by lo are ~1e-3-exact.

Layouts are precomputed on the host so every DMA is contiguous. All
weights stay resident in SBUF (54KB per partition; only the hi blocks
are DMAed -- the lo blocks are hi/32, derived by the otherwise-idle
DVE), x tiles stream per m-tile, and each [128, 512] output chunk
accumulates DPT matmuls before a DVE eviction fused with the bias
add. The first ST m-tiles run jointly, k-block-major, so PE
consumption paces the ~7 MB weight+startup preload; the leading DMA
groups are small so the first matmul starts early, and a burst of
throwaway matmuls on zeroed scratch warms the PE's HAM allocation
during the initial DMA wait.
"""

import sys
import types

import numpy as np

import concourse.mybir as mybir
import concourse.tile as tile
from concourse import bacc
from concourse.bass_utils import run_bass_kernel_spmd


def _ensure_axon_hooks():
    """run_bass_kernel_spmd(trace=True) (or BASS_TRACE=1 in the env) imports
    antenv.axon_hooks, which some agent images lack. Provide it, and register
    the ctypes NTFF hook if the boot shim is available, so tracing works (or
    degrades to a warning) instead of crashing."""
    try:
        import antenv.axon_hooks  # noqa: F401

        return
    except ImportError:
        pass
    m = types.ModuleType("antenv.axon_hooks")
    m._h = None
    m.set_axon_ntff_profile_hook = lambda h: setattr(m, "_h", h)
    m.get_axon_ntff_profile_hook = lambda: m._h
    sys.modules["antenv.axon_hooks"] = m
    try:
        import antenv

        antenv.axon_hooks = m
    except ImportError:
        pass
    try:
        from trn_agent_boot.trn_boot import _ntff_profile_via_ctypes

        m.set_axon_ntff_profile_hook(
            _ntff_profile_via_ctypes("/opt/axon/libaxon_pjrt.so")
        )
    except Exception:
        pass


_ensure_axon_hooks()

B, S, D_IN, D_OUT = 4, 2048, 4096, 4096
M_TOT = B * S  # 8192
N_CORES = 8
MG, OG = 2, 4  # data-parallel row groups x tensor-parallel out_feature groups
M_SH = M_TOT // MG  # 4096 rows per core
O_SH = D_OUT // OG  # 1024 out features per core
P = 128
HI_DP = D_IN // (2 * P)  # 16 hi pair-blocks of 256 contraction dims
LO_DP = 9  # lo pair-blocks (residual coverage of the first 2304 dims)
LO_K = LO_DP * 2 * P
DPT = HI_DP + LO_DP  # 27 pair-blocks in the fused stream
LO_SCALE = 32.0  # lo values x32, lo weights /32 (both exact in e4m3)
MT = M_SH // P  # 32 m-tiles per core
NF = 512  # psum free dim per matmul chunk (one PSUM bank of fp32)
NCH = O_SH // NF  # 2 output chunks per m-tile
ST = 3  # m-tiles processed jointly in the startup phase
GROUPS = [1, 2, 4, 4, 4, 4, 4, 2]  # x pair-blocks per startup DMA (sum = DPT)
assert sum(GROUPS) == DPT
WGROUPS = [1, 2, 4, 4, 5]  # hi weight pair-blocks per startup DMA (sum = 16)
assert sum(WGROUPS) == HI_DP
W2 = 2 * P  # free width of one x pair-block (h*128 + m)
WO = 2 * O_SH  # free width of one w pair-block (h*1024 + o)

_CACHE = {}


def _build():
    nc = bacc.Bacc("TRN2", target_bir_lowering=False, debug=False)
    f8, f32 = mybir.dt.float8e4, mybir.dt.float32

    # steady-state x, one m-tile per row: partition = d,
    # free = pb*256 + h*128 + m (DoubleRow pair layout)
    x_d = nc.dram_tensor("x8", [MT, P, DPT * W2], f8, kind="ExternalInput")
    # startup copies of m-tiles 0..ST-1, k-block-major, grouped for DMA:
    # free = pb*(ST*256) + st*256 + h*128 + m
    xs_d = nc.dram_tensor("xs8", [P, DPT * ST * W2], f8, kind="ExternalInput")
    # hi weights only: free = pb*2048 + h*1024 + o (lo weights = hi/32 are
    # derived on-chip by the otherwise-idle DVE)
    w_d = nc.dram_tensor("w8", [P, HI_DP * WO], f8, kind="ExternalInput")
    bias_d = nc.dram_tensor("biasb", [P, O_SH], f32, kind="ExternalInput")
    out_d = nc.dram_tensor("out", [M_SH, O_SH], f32, kind="ExternalOutput")

    DR = mybir.MatmulPerfMode.DoubleRow

    with tile.TileContext(nc) as tc:
        with (
            tc.tile_pool(name="wpool", bufs=1) as wpool,
            tc.tile_pool(name="xpool", bufs=4) as xpool,
            tc.tile_pool(name="psum", bufs=4, space="PSUM") as psum_pool,
        ):

            def load_x(mt, eng=None):
                x_t = xpool.tile([P, DPT * W2], f8, name="x", tag="x")
                (eng or nc.sync).dma_start(out=x_t[:], in_=x_d[mt])
                return x_t

            def alloc_psums():
                return [
                    psum_pool.tile([P, NF], f32, name=f"ps{oc}", tag=f"ps{oc}")
                    for oc in range(NCH)
                ]

            def mm(psums, x_pair, pb, oc):
                # x_pair: [P, 2, 128] fp8 pair view of one pair-block
                nc.tensor.matmul(
                    psums[oc][:],
                    x_pair,
                    w_sb[pb].rearrange("p (h o) -> p h o", h=2)[
                        :, :, oc * NF : (oc + 1) * NF
                    ],
                    start=pb == 0,
                    stop=pb == DPT - 1,
                    perf_mode=DR,
                )

            def evict(opool, mt, psums, ocs=None):
                for oc in ocs if ocs is not None else range(NCH):
                    o_sb = opool.tile([P, NF], f32, name="o_sb", tag=f"o{oc}")
                    nc.vector.tensor_add(
                        o_sb[:], psums[oc][:], bias_sb[:, oc * NF : (oc + 1) * NF]
                    )
                    nc.scalar.dma_start(
                        out=out_d[mt * P : (mt + 1) * P, oc * NF : (oc + 1) * NF],
                        in_=o_sb[:],
                    )

            w_sb = []  # per-pair-block [P, WO] views
            with tc.tile_pool(name="xstart", bufs=1) as xstart_pool:
                # HAM warmup: the PE's SBUF-port allocation ramps to 8/8 only
                # after several us of continuous activity (early matmuls issue
                # at ~534ns vs 220ns steady). Burn the initial DMA-wait window
                # with throwaway DoubleRow matmuls on zeroed scratch; they
                # land in psum set 0, which the real m-tile-0 group resets
                # via start=True.
                psums_st = [alloc_psums() for _ in range(ST)]
                warm = xstart_pool.tile([P, 2 * NF], f8, name="warm")
                nc.vector.memset(warm[:], 0)
                for _ in range(14):
                    nc.tensor.matmul(
                        psums_st[0][0][:],
                        warm[:, : 2 * P].rearrange("p (h m) -> p h m", h=2),
                        warm[:].rearrange("p (h o) -> p h o", h=2),
                        start=True,
                        stop=True,
                        perf_mode=DR,
                    )

                # startup x (m-tiles 0..ST-1) k-block-major plus the weight
                # stream, interleaved so each group lands as the PE needs it;
                # leading groups are small so the first matmul starts early
                # (xs before w: LDWEIGHTS loads the stationary x, so the xs
                # block gates the tensor queue ahead of the weights)
                xs_sb = []  # per-pair-block [P, ST*W2] views
                xoff = woff = 0
                wg = list(WGROUPS)
                for gi, gsz in enumerate(GROUPS):
                    t = xstart_pool.tile(
                        [P, gsz * ST * W2], f8, name=f"xs{gi}", tag=f"xs{gi}"
                    )
                    nc.sync.dma_start(
                        out=t[:],
                        in_=xs_d[:, xoff * ST * W2 : (xoff + gsz) * ST * W2],
                    )
                    for i in range(gsz):
                        xs_sb.append(t[:, i * ST * W2 : (i + 1) * ST * W2])
                    xoff += gsz
                    if wg:
                        wsz = wg.pop(0)
                        w = wpool.tile(
                            [P, wsz * WO], f8, name=f"w_{gi}", tag=f"w_{gi}"
                        )
                        nc.sync.dma_start(
                            out=w[:], in_=w_d[:, woff * WO : (woff + wsz) * WO]
                        )
                        for i in range(wsz):
                            w_sb.append(w[:, i * WO : (i + 1) * WO])
                        woff += wsz
                bias_sb = wpool.tile([P, O_SH], f32, name="bias_sb")
                nc.scalar.dma_start(out=bias_sb[:], in_=bias_d[:])

                # derive the lo weight blocks (hi/32, exact in e4m3) on the
                # DVE, which sits idle until the first evictions
                for lp in range(LO_DP):
                    wl = wpool.tile([P, WO], f8, name=f"wlo{lp}", tag=f"wlo{lp}")
                    nc.vector.tensor_scalar_mul(
                        wl[:], w_sb[lp], 1.0 / LO_SCALE
                    )
                    w_sb.append(wl)

                # prefetch steady-state x ahead of the startup evictions
                # (in-order sync stream: later dma_starts would head-of-line
                # block behind eviction DMAs otherwise)
                x_next = {mt: load_x(mt) for mt in range(ST, ST + 3)}

                # startup: ST m-tiles jointly, k-block-major, paced by the
                # weight stream
                for pb in range(DPT):
                    for st in range(ST):
                        xp = xs_sb[pb][
                            :, st * W2 : (st + 1) * W2
                        ].rearrange("p (h m) -> p h m", h=2)
                        for oc in range(NCH):
                            mm(psums_st[st], xp, pb, oc)

            with tc.tile_pool(name="opool", bufs=2) as opool:
                for st in range(ST):
                    evict(opool, st, psums_st[st])

                for mt in range(ST, MT):
                    x_t = x_next.pop(mt) if mt in x_next else load_x(mt)
                    psums = alloc_psums()
                    if mt < MT - 1:
                        for pb in range(DPT):
                            xp = x_t[
                                :, pb * W2 : (pb + 1) * W2
                            ].rearrange("p (h m) -> p h m", h=2)
                            for oc in range(NCH):
                                mm(psums, xp, pb, oc)
                        evict(opool, mt, psums)
                    else:
                        # last m-tile: oc-major so each output chunk finishes
                        # and evicts as early as possible; the final chunk
                        # drains in two half-width pieces to shorten the tail
                        for oc in range(NCH):
                            for pb in range(DPT):
                                xp = x_t[
                                    :, pb * W2 : (pb + 1) * W2
                                ].rearrange("p (h m) -> p h m", h=2)
                                mm(psums, xp, pb, oc)
                            if oc < NCH - 1:
                                evict(opool, mt, psums, ocs=[oc])
                            else:
                                for h in range(2):
                                    hf = NF // 2
                                    c0 = oc * NF + h * hf
                                    o_sb = opool.tile(
                                        [P, hf], f32, name="o_sb", tag=f"ot{h}"
                                    )
                                    nc.vector.tensor_add(
                                        o_sb[:],
                                        psums[oc][:, h * hf : (h + 1) * hf],
                                        bias_sb[:, c0 : c0 + hf],
                                    )
                                    # second piece on the idle sync ring so
                                    # the two final issues parallelize
                                    eng = nc.scalar if h == 0 else nc.sync
                                    eng.dma_start(
                                        out=out_d[
                                            mt * P : (mt + 1) * P, c0 : c0 + hf
                                        ],
                                        in_=o_sb[:],
                                    )
    nc.compile()
    return nc


def _prep_inputs(x, weight, bias):
    import ml_dtypes

    f8 = ml_dtypes.float8_e4m3
    x = np.asarray(x, dtype=np.float32)
    weight = np.asarray(weight, dtype=np.float32)
    bias = np.asarray(bias, dtype=np.float32)

    xf = np.ascontiguousarray(x.reshape(M_TOT, D_IN))
    x_hi = xf.astype(f8)
    res = xf - x_hi.astype(np.float32)
    x_lo = (res[:, :LO_K] * LO_SCALE).astype(f8)
    xcat = np.concatenate([x_hi, x_lo], axis=1)  # [M_TOT, DPT*256] f8

    qw = np.sign(weight)  # [o, d] f32

    # per o-group weights + broadcast bias, shared by cores in the group
    w_og, bias_og = [], []
    for og in range(OG):
        o0 = og * O_SH
        blk = np.ascontiguousarray(qw[o0 : o0 + O_SH, :].T)  # [d, o] f32
        # hi blocks only: [HI_DP, d, h*O_SH + o] -> grouped [P, HI_DP*WO]
        w8 = (
            blk.astype(f8)
            .reshape(HI_DP, 2, P, O_SH)
            .transpose(0, 2, 1, 3)
            .reshape(HI_DP, P, WO)
        )
        w_og.append(
            np.ascontiguousarray(w8.transpose(1, 0, 2)).reshape(P, HI_DP * WO)
        )
        bias_og.append(
            np.ascontiguousarray(
                np.broadcast_to(bias[o0 : o0 + O_SH], (P, O_SH))
            )
        )

    # per m-group x layouts, shared by cores in the group
    x_mg, xs_mg = [], []
    for mg in range(MG):
        m0 = mg * M_SH
        # steady state: [mt, d, pb*256 + h*128 + m]
        r = xcat[m0 : m0 + M_SH].reshape(MT, P, DPT, 2, P)  # [mt,m,pb,h,d]
        xt = np.ascontiguousarray(r.transpose(0, 4, 2, 3, 1)).reshape(
            MT, P, DPT * W2
        )
        x_mg.append(xt)
        # startup copies, k-block-major over the first ST m-tiles:
        # [pb, d, st*256 + h*128 + m] -> grouped [P, DPT*ST*W2]
        xs = np.empty((DPT, P, ST * W2), dtype=f8)
        for st in range(ST):
            xs[:, :, st * W2 : (st + 1) * W2] = (
                xt[st].reshape(P, DPT, W2).transpose(1, 0, 2)
            )
        xs_mg.append(
            np.ascontiguousarray(xs.transpose(1, 0, 2)).reshape(
                P, DPT * ST * W2
            )
        )

    in_maps = []
    for c in range(N_CORES):
        mg, og = c // OG, c % OG
        in_maps.append(
            {
                "x8": x_mg[mg],
                "xs8": xs_mg[mg],
                "w8": w_og[og],
                "biasb": bias_og[og],
            }
        )
    return in_maps


def run(inputs, trace=False):
    """Run the SPMD kernel; returns (full_output, BassKernelResults)."""
    if "nc" not in _CACHE:
        _CACHE["nc"] = _build()
    nc = _CACHE["nc"]
    in_maps = _prep_inputs(inputs["x"], inputs["weight"], inputs["bias"])
    res = run_bass_kernel_spmd(nc, in_maps, list(range(N_CORES)), trace=trace)
    out = np.empty((M_TOT, D_OUT), dtype=np.float32)
    for c in range(N_CORES):
        mg, og = c // OG, c % OG
        out[mg * M_SH : (mg + 1) * M_SH, og * O_SH : (og + 1) * O_SH] = res.results[
            c
        ]["out"]
    return out.reshape(B, S, D_OUT), res


def kernel(x, weight, bias):
    out, _ = run({"x": x, "weight": weight, "bias": bias})
    return out
